# revision 51
# baseline (speedup 1.0000x reference)
"""Trainium2 Bass kernel for nn_CrossAttention (B=4, N=4096, C1=320, C2=256, H=5).

Sharding: 8 independent (branch, batch) units -> one per NeuronCore.
Each core computes a full cross-attention branch for one batch element:
    q  = xq @ q_w.T            [N, 320], 5 heads x 64
    kv = xkv @ kv_w.T          [N, 640]
    y  = softmax(q k^T / 8) v  -> proj -> [N, 320]
Branch-2 inputs (C=256) are zero-padded to 320 host-side so all cores run
one SPMD program.

On-chip layout is feature-major (S^T formulation): K^T/Q^T live as
[feature, token] tiles so QK^T needs no transposes in the inner loop; the
softmax denominator comes from a ones-column appended to V; exp is the only
ScalarE op (no table switches).  All fp32 matmuls run as float32r (full PE
rate at moving>=256).  P and V are bf16 (error ~2e-3 relative).

trn2 matmuls tolerate at most ONE sync wait, so the emission discipline is:
- every matmul operand is produced on-chip by DVE/ACT (DMA loads are
  laundered through a DVE copy first);
- each PSUM-slot-claiming matmul is order-chained (sync=False edge) to the
  slot's previous claimer, so the bank WAW resolves by PE program order;
- tiny "absorber" matmuls (into a dedicated dummy PSUM bank, rotating
  cells) make PE observe the newest DVE tick before claiming a slot whose
  previous readers were DVE copies.
"""

import numpy as np

import concourse.bass as bass
import concourse.bacc as bacc
import concourse.mybir as mybir
import concourse.tile as tile
from concourse.tile import add_dep_helper

F32 = mybir.dt.float32
F32R = mybir.dt.float32r
BF16 = mybir.dt.bfloat16
EXP = mybir.ActivationFunctionType.Exp

B, N, C1, C2, H = 4, 4096, 320, 256, 5
HD = 64
SCALE = HD ** -0.5
C = 320          # padded input feature dim (both branches)
CK = 2 * C1      # kv projection output dim (640)
KC = 128         # key chunk (tokens per S^T partition tile)
QT = 512         # query tile (moving free dim)
G = 3            # key chunks per exp group (PSUM: 3 banks x 2 bufs)

CCH = [(0, 128), (128, 128), (256, 64)]   # contraction chunks over 320
FCH = [(0, 128), (128, 128), (256, 64)]   # output-feature tiles over 320
PCH = [(i * 64, 64) for i in range(5)]    # per-head chunks over 320


def build(nc: bass.Bass, n: int = N, repeat: int = 1):
    nq = n // QT
    nkc = n // KC
    VW = H * (HD + 1)  # 325

    xq_d = nc.dram_tensor("xq", [n, C], F32, kind="ExternalInput").ap()
    xkv_d = nc.dram_tensor("xkv", [n, C], F32, kind="ExternalInput").ap()
    qw_d = nc.dram_tensor("qw", [C1, C], F32, kind="ExternalInput").ap()
    kvw_d = nc.dram_tensor("kvw", [CK, C], F32, kind="ExternalInput").ap()
    pw_d = nc.dram_tensor("pw", [C1, C1], F32, kind="ExternalInput").ap()
    pb_d = nc.dram_tensor("pb", [1, C1], F32, kind="ExternalInput").ap()
    id_d = nc.dram_tensor("ident", [128, 128], F32, kind="ExternalInput").ap()
    on_d = nc.dram_tensor("ones", [1, 128], F32, kind="ExternalInput").ap()
    y_d = nc.dram_tensor("y", [n, C1], F32, kind="ExternalOutput").ap()

    with tile.TileContext(nc) as tc:
        from contextlib import ExitStack
        ctx = ExitStack()
        with ctx:
            const = ctx.enter_context(tc.tile_pool(name="const", bufs=1))
            stage = ctx.enter_context(tc.tile_pool(name="stage", bufs=4))
            work = ctx.enter_context(tc.tile_pool(name="work", bufs=2))
            ppool = ctx.enter_context(tc.tile_pool(name="ppool", bufs=2))
            ps_pv = ctx.enter_context(tc.tile_pool(name="ps_pv", bufs=1, space="PSUM"))
            ps_m = ctx.enter_context(tc.tile_pool(name="ps_m", bufs=1, space="PSUM"))

            # ---------------- emission-discipline helpers ----------------
            state = {"absorb": 0, "misc": 0, "setup": 0,
                     "last_dve": None, "last_dve_inst": None,
                     "absorb_bank": None}
            slot_last = {}

            def dve_chain(inst):
                # Linearize DVE in emission order so its semaphore ticks are
                # monotone w.r.t. emission; then one absorber covers every
                # earlier DVE product.
                prev = state["last_dve_inst"]
                if prev is not None:
                    add_dep_helper(inst.ins, prev.ins, sync=False,
                                   reason="linearize DVE")
                state["last_dve_inst"] = inst

            def dve_copy(dst, src, track=None):
                dve_chain(nc.vector.tensor_copy(dst, src))
                state["last_dve"] = dst if track is None else track

            def absorb(ap=None):
                """Tiny matmul reading ap (or the newest DVE product): PE
                observes the producer's semaphore tick so the next real
                matmul needs at most one sync wait. Output cells rotate
                through cols 480..511 of the current misc/setup PSUM bank and
                are chained into that bank's claim order (PE is in-order, so
                the garbage cell is always overwritten before anyone reads)."""
                if ap is None:
                    ap = state["last_dve"]
                ap11 = ap[0:1, 0:1]
                if ap11.dtype == F32R:
                    ap11 = ap11.bitcast(F32)
                i = state["absorb"]
                state["absorb"] += 1
                c = 480 + i % 32
                bank, slot = state["absorb_bank"]()
                mm = nc.tensor.matmul(
                    bank[0:1, c : c + 1], ap11, ap11,
                    start=True, stop=True, skip_group_check=True,
                )
                chain(mm, [mm], slot)
                return mm

            def misc_tile():
                i = state["misc"]
                state["misc"] += 1
                t = ps_m.tile([128, 512], F32, tag="misc", name=f"misc{i}")
                return t, ("misc", 0)

            def chain(mm_first, group, slot):
                """Order a PSUM-slot claim after every matmul of the slot's
                previous claim group (PE in-order => bank WAW needs no
                semaphore; edges to all members because the scheduler may
                reorder within a group)."""
                for prev in slot_last.get(slot, ()):
                    add_dep_helper(mm_first.ins, prev.ins, sync=False,
                                   reason="order psum slot claim")
                slot_last[slot] = list(group)

            # ---------------- persistent tiles ---------------------------
            pv_bank = ps_pv.tile([128, QT], F32, name="pv_bank")
            ident = const.tile([128, 128], F32)
            ones_r = const.tile([1, 128], F32R)
            bias_r = const.tile([1, C1], F32R)
            qwT = [const.tile([128, C1], F32R, tag=f"qwT{j}", name=f"qwT{j}") for j in range(3)]
            kwT = [const.tile([128, CK], F32R, tag=f"kwT{j}", name=f"kwT{j}") for j in range(3)]
            pwT = [const.tile([64, C1], F32R, tag=f"pwT{j}", name=f"pwT{j}") for j in range(5)]
            ktile = [const.tile([128, n], F32R, tag=f"kt{j}", name=f"kt{j}") for j in range(3)]
            v_sb = const.tile([128, nkc * VW], BF16)

            # ---------------- constants ----------------------------------
            ist = stage.tile([128, 128], F32, tag="ist")
            nc.sync.dma_start(ist[:], id_d[:])
            dve_copy(ident[:], ist[:])
            ost = stage.tile([1, 128], F32, tag="ost")
            nc.sync.dma_start(ost[:], on_d[:])
            dve_copy(ones_r[:], ost[:])
            bst = stage.tile([1, C1], F32, tag="bst")
            nc.sync.dma_start(bst[:], pb_d[:])
            dve_copy(bias_r[:], bst[:])

            v_ones = v_sb[:].rearrange("p (c h x) -> p c h x", h=H, x=HD + 1)[
                :, :, :, HD : HD + 1
            ]
            dve_chain(nc.vector.memset(v_ones, 1.0))

            def preamble_steps(qt):
                xqT = [work.tile([128, 512], F32R, tag=f"xqT{j}", name=f"xqT{j}_{qt}") for j in range(3)]
                qT = [work.tile([128, QT], F32R, tag=f"qT{j}", name=f"qT{j}_{qt}") for j in range(3)]
                steps = []

                def ts_step(ts):
                    t0 = qt * 512 + ts * 128
                    qst = stage.tile([128, C], F32, tag="qst")
                    nc.sync.dma_start(qst[:], xq_d[t0 : t0 + 128, :])
                    qst2 = stage.tile([128, C], F32, tag="qst2")
                    dve_copy(qst2[:], qst[:])
                    absorb(qst2)
                    for cj, (c0, cs) in enumerate(CCH):
                        tp, slot = misc_tile()
                        t = nc.tensor.transpose(
                            tp[0:cs, 0:128], qst2[:, c0 : c0 + cs], ident[:]
                        )
                        chain(t, [t], slot)
                        dve_copy(xqT[cj][0:cs, ts * 128 : ts * 128 + 128], tp[0:cs, 0:128])

                for ts in range(4):
                    steps.append(lambda ts=ts: ts_step(ts))

                def q_step():
                    absorb()  # newest xqT copy
                    for ft, (f0, fs) in enumerate(FCH):
                        qp, slot = misc_tile()
                        grp = []
                        for cj, (c0, cs) in enumerate(CCH):
                            grp.append(nc.tensor.matmul(
                                qp[0:fs, :],
                                qwT[cj][0:cs, f0 : f0 + fs],
                                xqT[cj][0:cs, :],
                                start=(cj == 0),
                                stop=(cj == 2),
                                skip_group_check=True,
                            ))
                        chain(grp[0], grp, slot)
                        dve_copy(qT[ft][0:fs, :], qp[0:fs, :])

                steps.append(q_step)
                return qT, steps

            def preamble(qt):
                qT, steps = preamble_steps(qt)
                for s in steps:
                    s()
                return qT

            # ---------------- setup: weights + K^T + V -------------------
            # dedicated wide PSUM pool; exits before the attention pools
            with tc.tile_pool(name="ps_setup", bufs=6, space="PSUM") as ps_setup:

                def setup_tile():
                    i = state["setup"]
                    state["setup"] += 1
                    t = ps_setup.tile([128, 512], F32, tag="st", name=f"st{i}")
                    return t, ("st", i % 6)

                state["absorb_bank"] = setup_tile

                def transpose_to(dst_ap, src_ap, fs, cs):
                    tp, slot = setup_tile()
                    t = nc.tensor.transpose(
                        tp[0:cs, 0:fs], src_ap, ident[0:fs, 0:fs]
                    )
                    chain(t, [t], slot)
                    dve_copy(dst_ap, tp[0:cs, 0:fs])

                def load_transposed(w_d, rows, dsts, chunks):
                    nrt = (rows + 127) // 128
                    for fi in range(nrt):
                        f0, fs = fi * 128, min(128, rows - fi * 128)
                        wst = stage.tile([128, C], F32, tag="wst")
                        nc.sync.dma_start(wst[0:fs, :], w_d[f0 : f0 + fs, :])
                        wst2 = stage.tile([128, C], F32, tag="wst2")
                        dve_copy(wst2[0:fs, :], wst[0:fs, :])
                        absorb(wst2)
                        for cj, (c0, cs) in enumerate(chunks):
                            transpose_to(
                                dsts[cj][0:cs, f0 : f0 + fs],
                                wst2[0:fs, c0 : c0 + cs],
                                fs, cs,
                            )

                load_transposed(qw_d, C1, qwT, CCH)
                load_transposed(kvw_d, CK, kwT, CCH)
                load_transposed(pw_d, C1, pwT, PCH)

                for nb in range(n // 512):
                    xkvT = [work.tile([128, 512], F32R, tag=f"xkvT{j}", name=f"xkvT{j}_{nb}") for j in range(3)]
                    for ts in range(4):
                        t0 = nb * 512 + ts * 128
                        kst = stage.tile([128, C], F32, tag="kst")
                        nc.sync.dma_start(kst[:], xkv_d[t0 : t0 + 128, :])
                        kst2 = stage.tile([128, C], F32, tag="kst2")
                        dve_copy(kst2[:], kst[:])
                        absorb(kst2)
                        for cj, (c0, cs) in enumerate(CCH):
                            transpose_to(
                                xkvT[cj][0:cs, ts * 128 : ts * 128 + 128],
                                kst2[:, c0 : c0 + cs],
                                128, cs,
                            )
                    absorb()  # newest xkvT copy
                    for ft, (f0, fs) in enumerate(FCH):
                        kp, slot = setup_tile()
                        grp = []
                        for cj, (c0, cs) in enumerate(CCH):
                            grp.append(nc.tensor.matmul(
                                kp[0:fs, :],
                                kwT[cj][0:cs, f0 : f0 + fs],
                                xkvT[cj][0:cs, :],
                                start=(cj == 0),
                                stop=(cj == 2),
                                skip_group_check=True,
                            ))
                        chain(grp[0], grp, slot)
                        dve_copy(ktile[ft][0:fs, nb * 512 : (nb + 1) * 512], kp[0:fs, :])
                    for ts in range(4):
                        absorb()
                        vp, slot = setup_tile()
                        grp = []
                        for cj, (c0, cs) in enumerate(CCH):
                            grp.append(nc.tensor.matmul(
                                vp[0:128, 0:C1],
                                xkvT[cj][0:cs, ts * 128 : ts * 128 + 128],
                                kwT[cj][0:cs, C1:CK],
                                start=(cj == 0),
                                stop=(cj == 2),
                                skip_group_check=True,
                            ))
                        chain(grp[0], grp, slot)
                        blk = (nb * 4 + ts) * VW
                        vdst = v_sb[:, blk : blk + VW].rearrange(
                            "p (h x) -> p h x", x=HD + 1
                        )[:, :, 0:HD]
                        vsrc = vp[:, 0:C1].rearrange("p (h x) -> p h x", x=HD)
                        dve_copy(vdst, vsrc, track=v_sb[:, blk : blk + HD])

            # ---------------- attention pools ----------------------------
            ps_s = ctx.enter_context(tc.tile_pool(name="ps_s", bufs=2, space="PSUM"))
            state["absorb_bank"] = misc_tile

            ngroups = (nkc + G - 1) // G
            prev_bmm = None
            pending = []
            pre_qT = preamble(0)
            for rep_qt in range(nq * repeat):
                qt = rep_qt % nq
                qT = pre_qT
                if rep_qt + 1 < nq * repeat:
                    pre_qT, nsteps = preamble_steps((rep_qt + 1) % nq)
                    pending.extend(nsteps)

                absorb()  # newest DVE product covers qT copies
                oT = [work.tile([HD, QT], F32R, tag=f"oT{h}", name=f"oT{h}_{qt}") for h in range(H)]

                # cross-head linearized S/exp/PV pipeline: exp(i) overlaps
                # PV(i-1)/S(i+1), including across head boundaries, so
                # ScalarE (the bottleneck) never idles
                groups = [(h, gg) for h in range(H) for gg in range(ngroups)]
                sps = {}
                pv_state = {"first": None, "grp": [], "prev_mm": None}
                prev_exp = {}

                def emit_S(i):
                    h, gg = groups[i]
                    kt, ko = h // 2, (h % 2) * 64
                    g0, g1 = gg * G, min((gg + 1) * G, nkc)
                    sp = ps_s.tile([128, G * QT], F32, tag="s", name=f"s_{qt}_{h}_{gg}")
                    sps[i] = sp
                    sgrp = []
                    for j, kc in enumerate(range(g0, g1)):
                        sgrp.append(nc.tensor.matmul(
                            sp[:, j * QT : (j + 1) * QT],
                            ktile[kt][ko : ko + 64, kc * KC : (kc + 1) * KC],
                            qT[kt][ko : ko + 64, :],
                            start=True,
                            stop=True,
                        ))
                    for smm in sgrp[1:]:
                        add_dep_helper(smm.ins, sgrp[0].ins, sync=False,
                                       reason="order S group")
                    if pv_state["prev_mm"] is not None:
                        add_dep_helper(sgrp[0].ins, pv_state["prev_mm"].ins, sync=False,
                                       reason="order S(i+2) after PV(i)")
                    chain(sgrp[0], sgrp, ("s", i % 2))

                def finish_head(h):
                    nonlocal prev_bmm
                    chain(pv_state["first"], pv_state["grp"], ("pv", 0))
                    if prev_bmm is not None:
                        add_dep_helper(
                            pv_state["first"].ins, prev_bmm.ins, sync=False,
                            reason="order PV after prev bcast",
                        )
                    pv_state["first"] = None
                    pv_state["grp"] = []
                    pvc = ppool.tile([65, QT], F32, tag="pvc", name=f"pvc_{qt}_{h}")
                    dve_copy(pvc[:], pv_bank[0:65, :])
                    din = ppool.tile([1, QT], F32, tag="din", name=f"din_{qt}_{h}")
                    with nc.allow_low_precision(reason="softmax denom reciprocal"):
                        dve_chain(nc.vector.reciprocal(din[:], pvc[64:65, :]))
                    state["last_dve"] = din
                    # denom broadcast on the idle Pool engine (frees PE);
                    # absorber keeps the next head's PV frontier at one wait
                    bcs = ppool.tile([64, QT], F32, tag="bcs", name=f"bcs_{qt}_{h}")
                    nc.gpsimd.partition_broadcast(bcs[:], din[:], channels=64)
                    prev_bmm = absorb(din)
                    dve_chain(nc.vector.tensor_mul(oT[h][:], pvc[0:64, :], bcs[:]))
                    state["last_dve"] = oT[h]

                emit_S(0)
                if len(groups) > 1:
                    emit_S(1)
                for i, (h, gg) in enumerate(groups):
                    g0, g1 = gg * G, min((gg + 1) * G, nkc)
                    cw = (g1 - g0) * QT
                    pt = ppool.tile([128, G * QT], BF16, tag="pt", name=f"pt_{qt}_{h}_{gg}")
                    einst = nc.scalar.activation(pt[:, 0:cw], sps.pop(i)[:, 0:cw], EXP, scale=SCALE)
                    if (i % 2) in prev_exp:
                        add_dep_helper(einst.ins, prev_exp[i % 2].ins, sync=False,
                                       reason="order exp pt-slot claim (ACT in-order)")
                    prev_exp[i % 2] = einst
                    for j, kc in enumerate(range(g0, g1)):
                        blk = kc * VW
                        mm = nc.tensor.matmul(
                            pv_bank[0:65, :],
                            v_sb[:, blk + h * (HD + 1) : blk + (h + 1) * (HD + 1)],
                            pt[:, j * QT : (j + 1) * QT],
                            start=(kc == 0),
                            stop=(kc == nkc - 1),
                            skip_group_check=True,
                        )
                        if pv_state["first"] is None:
                            pv_state["first"] = mm
                        pv_state["grp"].append(mm)
                        pv_state["prev_mm"] = mm
                    if i + 2 < len(groups):
                        emit_S(i + 2)
                    if pending and i % 6 == 5:
                        pending.pop(0)()
                    if gg == ngroups - 1:
                        finish_head(h)

                while pending:
                    pending.pop(0)()
                # projection + bias, token-major output
                for ts in range(4):
                    absorb()
                    pp, slot = misc_tile()
                    grp = [nc.tensor.matmul(
                        pp[0:128, 0:C1], ones_r[:], bias_r[:],
                        start=True, stop=False, skip_group_check=True,
                    )]
                    for h in range(H):
                        grp.append(nc.tensor.matmul(
                            pp[0:128, 0:C1],
                            oT[h][:, ts * 128 : (ts + 1) * 128],
                            pwT[h][0:64, :],
                            start=False,
                            stop=(h == H - 1),
                            skip_group_check=True,
                        ))
                    chain(grp[0], grp, slot)
                    ysb = ppool.tile([128, C1], F32, tag="ysb", name=f"ysb_{qt}_{ts}")
                    dve_copy(ysb[:], pp[0:128, 0:C1])
                    t0 = qt * 512 + ts * 128
                    nc.sync.dma_start(y_d[t0 : t0 + 128, :], ysb[:])


def _pad_cols(w, cols: int) -> np.ndarray:
    w = np.asarray(w, np.float32)
    if w.shape[-1] == cols:
        return np.ascontiguousarray(w)
    out = np.zeros(w.shape[:-1] + (cols,), np.float32)
    out[..., : w.shape[-1]] = w
    return out


_CACHE = {}


def kernel(x1, x2, q1_w, q2_w, kv1_w, kv2_w, proj1_w, proj1_b, proj2_w, proj2_b):
    from concourse import bass_utils

    x1 = np.asarray(x1, np.float32)
    x2p = _pad_cols(x2, C)

    ident = np.eye(128, dtype=np.float32)
    ones = np.ones((1, 128), np.float32)
    base1 = {
        "qw": np.ascontiguousarray(np.asarray(q1_w, np.float32)),
        "kvw": _pad_cols(kv1_w, C),
        "pw": np.ascontiguousarray(np.asarray(proj1_w, np.float32)),
        "pb": np.asarray(proj1_b, np.float32).reshape(1, C1),
        "ident": ident,
        "ones": ones,
    }
    base2 = {
        "qw": _pad_cols(q2_w, C),
        "kvw": np.ascontiguousarray(np.asarray(kv2_w, np.float32)),
        "pw": np.ascontiguousarray(np.asarray(proj2_w, np.float32)),
        "pb": np.asarray(proj2_b, np.float32).reshape(1, C1),
        "ident": ident,
        "ones": ones,
    }
    in_maps = []
    for b in range(B):
        in_maps.append(
            dict(base1, xq=np.ascontiguousarray(x1[b]), xkv=np.ascontiguousarray(x2p[b]))
        )
    for b in range(B):
        in_maps.append(
            dict(base2, xq=np.ascontiguousarray(x2p[b]), xkv=np.ascontiguousarray(x1[b]))
        )

    if "nc" not in _CACHE:
        nc = bacc.Bacc("TRN2", target_bir_lowering=False, debug=False, num_devices=8)
        build(nc)
        nc.compile()
        _CACHE["nc"] = nc
    nc = _CACHE["nc"]

    res = bass_utils.run_bass_kernel_spmd(nc, in_maps, core_ids=list(range(8)))
    out = np.empty((B, N, 2 * C1), np.float32)
    for b in range(B):
        out[b, :, :C1] = res.results[b]["y"]
        out[b, :, C1:] = res.results[B + b]["y"]
    return out
